# revision 52
# baseline (speedup 1.0000x reference)
"""Causal single-head attention (B=8, T=2048, D=128, H=16) on 8 Trainium2 cores.

Strategy: data-parallel over batch (1 batch element per NeuronCore). Per core:
  - x arrives pre-transposed from the host as xT [D, T] (contiguous DMA).
  - Project qT/kT = (Wq/Wk)^T @ xT with head dim zero-padded 16->128 so every
    matmul contracts over K=128; v tiles [128, 17] carry a ones column so the
    softmax denominator falls out of the PV matmul for free.
  - Scores are computed TRANSPOSED: ST[keys, queries] = kT_j^T @ qT_block, so
    exp(ST) (ACT, scale=1/4 folded in) is directly the PV stationary-side
    operand -- no per-tile transposes of the probability matrix.
  - PV: O'T[17, W] += V'_j^T @ PT_j accumulated in PSUM over key tiles.
  - Causal masking: only key tiles j with 128*j < W*(qb+1) are computed; the
    two diagonal tiles get a multiplicative 0/1 mask after exp.
  - ST-matmul groups are software-pipelined up to 4 groups ahead of the PV
    matmuls (tapered at the end) so TensorE streams future scores while
    ScalarE exponentiates and TensorE consumes finished groups for PV;
    ScalarE (the exp bottleneck, ~1 elem/lane/cycle) stays saturated.
Output per core: outT [17, T] (16 unnormalized head dims + the exp-sum row).
Host divides and transposes during the gather step.
"""

import os

import numpy as np

B, T, D, H = 8, 2048, 128, 16
NT = T // 128        # 16 key tiles of 128
W = 256              # query block width (fp32r needs moving dim >= 256)
NQB = T // W         # 8 query blocks
GROUP = 4            # key tiles per exp call ([128, GROUP*W] = 2 PSUM banks)
SCALE = H ** -0.5

_CACHE = {}


def _build(prec: str):
    import concourse.mybir as mybir
    import concourse.tile as tile
    from concourse import bacc

    f32 = mybir.dt.float32
    mm_dt = {"f32r": mybir.dt.float32r, "f16": mybir.dt.float16, "f32": f32}[prec]
    Exp = mybir.ActivationFunctionType.Exp

    nc = bacc.Bacc()
    xT_d = nc.declare_dram_parameter("xT", [D, T], mm_dt, isOutput=False)
    # packed constants: wq[0:128] | wk[128:256] | wv[256:272]
    cst = nc.declare_dram_parameter("cst", [128, 272], mm_dt, isOutput=False)
    outT = nc.declare_dram_parameter("outT", [H + 1, T], f32, isOutput=True)

    with tile.TileContext(nc) as tc:
        with tc.tile_pool(name="sb", bufs=1) as sb:
            # ---- persistent SBUF buffers ----
            cst_sb = sb.tile([128, 272], mm_dt, tag="cst")
            nc.gpsimd.dma_start(cst_sb[:], cst.ap())  # SWDGE: parallel to x
            wq_sb = cst_sb[:, 0:128]
            wk_sb = cst_sb[:, 128:256]
            wv_sb = cst_sb[:, 256:272]
            # diagonal masks generated on the idle GPSIMD engine:
            # dm[:, c] over the two diagonal key tiles (see _host_inputs docs)
            mdt = mm_dt if prec == "f16" else f32
            dm_full = sb.tile([128, 2 * W], mdt, tag="dm")
            nc.gpsimd.memset(dm_full[:], 1.0)
            nc.gpsimd.affine_select(
                out=dm_full[:, :W], in_=dm_full[:, :W],
                compare_op=mybir.AluOpType.is_ge, fill=0.0,
                base=0, pattern=[[1, W]], channel_multiplier=-1,
            )
            nc.gpsimd.affine_select(
                out=dm_full[:, W:], in_=dm_full[:, W:],
                compare_op=mybir.AluOpType.is_ge, fill=0.0,
                base=-128, pattern=[[1, W]], channel_multiplier=-1,
            )
            dm_sb = dm_full

            CH = [(0, 256), (256, 256), (512, 512), (1024, 512), (1536, 512)]
            xT = sb.tile([128, T], mm_dt, tag="xT")           # [d, t]
            for c0, cw in CH:
                nc.sync.dma_start(
                    xT[:, c0:c0 + cw], xT_d.ap()[:, c0:c0 + cw]
                )

            warm = sb.tile([1, 2], f32, tag="warm")
            nc.vector.memset(warm[:, 0:1], 0.0)
            nc.scalar.activation(warm[:, 1:2], warm[:, 0:1], Exp)
            # PE warm-up: dummy matmuls during the input DMA keep the HAM
            # activity monitor busy so real matmuls start at full clock.
            wdum = sb.tile([128, 512], f32, tag="wdum")
            nc.vector.memset(wdum[:], 0.0)

            qTc = [sb.tile([128, cw], mm_dt, tag=f"qT{g}", name=f"qT{g}")
                   for g, (c0, cw) in enumerate(CH)]
            kTc = [sb.tile([128, cw], mm_dt, tag=f"kT{g}", name=f"kT{g}")
                   for g, (c0, cw) in enumerate(CH)]
            vSc = [sb.tile([128, cw // 128, H + 1], mm_dt, tag=f"vS{g}", name=f"vS{g}")
                   for g, (c0, cw) in enumerate(CH)]

            def chunk_of(col):  # chunk index, offset for column `col`
                for g, (c0, cw) in enumerate(CH):
                    if c0 <= col < c0 + cw:
                        return g, col - c0
                raise AssertionError(col)
            oTc = [sb.tile([H + 1, W], f32, tag=f"oT{qb}", name=f"oT{qb}") for qb in range(NQB)]

            groups = []
            for qb in range(NQB):
                nj = (W * (qb + 1)) // 128
                qb_groups = [
                    (qb, nj, g0, min(GROUP, nj - g0))
                    for g0 in range(0, nj, GROUP)
                ]
                # descending start: the diagonal (masked) group is consumed
                # first, keeping the exp->mask->PV chain off the qb tail
                groups.extend(reversed(qb_groups))

            with (
                tc.tile_pool(name="psS", bufs=3, space="PSUM") as psS,
                tc.tile_pool(name="psO", bufs=2, space="PSUM") as psO,
                tc.tile_pool(name="pt", bufs=8) as ptp,
            ):
                o_tiles = {}
                pt_tiles = {}

                def emit_proj(g):
                    c0, cw = CH[g]
                    nt = cw // 128
                    sl = slice(c0, c0 + cw)
                    if 2 * cw + nt * H <= GROUP * W:
                        pp = psS.tile([128, GROUP * W], f32, tag="st", name=f"pp{g}")
                        pk, pq, pv = pp[:, :cw], pp[:, cw:2 * cw], pp[:, 2 * cw:2 * cw + nt * H]
                    else:
                        pp = psS.tile([128, GROUP * W], f32, tag="st", name=f"ppa{g}")
                        pp2 = psS.tile([128, GROUP * W], f32, tag="st", name=f"ppb{g}")
                        pk, pv = pp[:, :cw], pp[:, cw:cw + nt * H]
                        pq = pp2[:, :cw]
                    nc.tensor.matmul(pk, wk_sb[:], xT[:, sl])
                    if g <= 2:  # ramp phase: ACT has idle capacity
                        nc.scalar.copy(kTc[g][:], pk)
                    else:
                        nc.vector.tensor_copy(kTc[g][:], pk)
                    nc.tensor.matmul(pq, wq_sb[:], xT[:, sl])
                    nc.vector.tensor_copy(qTc[g][:], pq)
                    pvv = pv.rearrange("p (n h) -> p n h", n=nt)
                    for u in range(nt):
                        i = (c0 // 128) + u
                        nc.tensor.matmul(
                            pvv[:, u, :], xT[:, 128 * i:128 * (i + 1)], wv_sb[:]
                        )
                    nc.vector.tensor_copy(vSc[g][:, :, :H], pvv[:])
                    if prec == "f32r":
                        nc.vector.memset(vSc[g][:, :, H].bitcast(f32), 1.0)
                    else:
                        nc.vector.memset(vSc[g][:, :, H], 1.0)

                def q_ap(qb):
                    g, off = chunk_of(W * qb)
                    return qTc[g][:, off:off + W]

                def emit_st_exp(idx):
                    qb, nj, g0, gn = groups[idx]
                    st = psS.tile([128, GROUP * W], f32, tag="st")
                    for jj in range(gn):
                        j = g0 + jj
                        kg, koff = chunk_of(128 * j)
                        nc.tensor.matmul(
                            st[:, jj * W:(jj + 1) * W],
                            kTc[kg][:, koff:koff + 128],
                            q_ap(qb),
                        )
                    pt = ptp.tile([128, GROUP * W], mm_dt, tag="pt")
                    pt_tiles[idx] = pt
                    nc.scalar.activation(
                        pt[:, :gn * W], st[:, :gn * W], Exp, scale=SCALE
                    )
                    if g0 + gn == nj:  # group holding the 2 diagonal tiles
                        off = (nj - 2 - g0) * W
                        nc.vector.tensor_mul(
                            pt[:, off:off + 2 * W], pt[:, off:off + 2 * W], dm_sb[:]
                        )

                def emit_pv(idx):
                    qb, nj, g0, gn = groups[idx]
                    first_emitted = g0 + GROUP >= nj   # diag group comes first
                    last_emitted = g0 == 0
                    if first_emitted:
                        o_tiles[qb] = psO.tile([H + 1, W], f32, tag="o", name=f"o{qb}")
                    pt = pt_tiles.pop(idx)
                    for jj in range(gn):
                        j = g0 + jj
                        vg, voff = chunk_of(128 * j)
                        nc.tensor.matmul(
                            o_tiles[qb][:],
                            vSc[vg][:, voff // 128, :],
                            pt[:, jj * W:(jj + 1) * W],
                            start=(first_emitted and jj == 0),
                            stop=(last_emitted and jj == gn - 1),
                        )
                    if last_emitted:
                        nc.vector.tensor_copy(oTc[qb][:], o_tiles.pop(qb)[:])
                        nc.sync.dma_start(
                            outT.ap()[:, W * qb:W * (qb + 1)], oTc[qb][:]
                        )

                # emission plan: projection chunks land just before the
                # first query block that needs them; ST/exp runs up to 4
                # groups ahead of PV (tapered near the end).
                # assign each proj chunk to the first group that needs it
                # projection-chunk emission slots (group indices). Chunks
                # MUST be emitted at or before the first group that reads
                # them: Tile tracks dependencies by trace order, so a
                # consumer emitted before its producer silently reads stale
                # SBUF (verified: gives nondeterministic garbage).
                proj_at = {0: [0, 1], 1: [2], 2: [3], 6: [4]}
                first_need = {}
                for i2, (qb, nj, g0, gn) in enumerate(groups):
                    need = {chunk_of(W * qb)[0]}
                    need.update(chunk_of(128 * j)[0] for j in range(g0, g0 + gn))
                    for g in need:
                        first_need.setdefault(g, i2)
                for slot, gs in proj_at.items():
                    for g in gs:
                        assert slot <= first_need[g], (slot, g, first_need[g])

                n = len(groups)
                pdum = psS.tile([128, GROUP * W], f32, tag="st", name="pdum")
                for r in range(4):
                    nc.tensor.matmul(
                        pdum[:, :512], wdum[:, :128].bitcast(mm_dt),
                        wdum[:].bitcast(mm_dt),
                    )
                for g in proj_at.pop(0, []):
                    emit_proj(g)
                pend = []
                for idx in range(n):
                    for g in proj_at.pop(idx, []):
                        emit_proj(g)
                    emit_st_exp(idx)
                    pend.append(idx)
                    depth = 4 if idx < n - 4 else max(1, n - 1 - idx)
                    while len(pend) > depth:
                        emit_pv(pend.pop(0))
                while pend:
                    emit_pv(pend.pop(0))

    nc.finalize()
    return nc


def _get_nc(prec: str):
    if prec not in _CACHE:
        _CACHE[prec] = _build(prec)
    return _CACHE[prec]


def _host_inputs(Wq, Wk, Wv):
    Wq, Wk, Wv = (np.asarray(w, dtype=np.float32) for w in (Wq, Wk, Wv))
    cst = np.zeros((128, 272), np.float32)
    cst[:, 0:H] = Wq
    cst[:, 128:128 + H] = Wk
    cst[:D, 256:256 + H] = Wv
    return cst


def kernel(inpEmb, Wq, Wk, Wv):
    from concourse.bass_utils import run_bass_kernel_spmd

    prec = os.environ.get("ATT_PREC", "f32r")
    nc = _get_nc(prec)
    np_dt = np.float16 if prec == "f16" else np.float32
    cst = _host_inputs(Wq, Wk, Wv).astype(np_dt)
    x = np.asarray(inpEmb, dtype=np.float32)
    in_maps = [
        {"xT": np.ascontiguousarray(x[b].T.astype(np_dt)), "cst": cst}
        for b in range(B)
    ]
    def run_and_check():
        br = run_bass_kernel_spmd(nc, in_maps, list(range(B)))
        out = np.empty((B, T, H), np.float32)
        for b in range(B):
            oT = br.results[b]["outT"]
            sums = oT[H]
            # softmax denominators are sums of exponentials: must be finite
            # and strictly positive; anything else means the device run was
            # bad (unwritten/partial output) and should be retried.
            if not (np.isfinite(oT).all() and (sums > 0.0).all()):
                raise RuntimeError(f"core {b}: invalid kernel output")
            out[b] = (oT[:H] / sums[None, :]).T
        return out

    for attempt in range(3):
        try:
            return run_and_check()
        except Exception:
            if attempt == 2:
                raise


# revision 60
# speedup vs baseline: 1.0043x; 1.0043x over previous
"""Causal single-head attention (B=8, T=2048, D=128, H=16) on 8 Trainium2 cores.

Strategy: data-parallel over batch (1 batch element per NeuronCore). Per core:
  - x arrives pre-transposed from the host as xT [D, T] (contiguous DMA).
  - Project qT/kT = (Wq/Wk)^T @ xT with head dim zero-padded 16->128 so every
    matmul contracts over K=128; v tiles [128, 17] carry a ones column so the
    softmax denominator falls out of the PV matmul for free.
  - Scores are computed TRANSPOSED: ST[keys, queries] = kT_j^T @ qT_block, so
    exp(ST) (ACT, scale=1/4 folded in) is directly the PV stationary-side
    operand -- no per-tile transposes of the probability matrix.
  - PV: O'T[17, W] += V'_j^T @ PT_j accumulated in PSUM over key tiles.
  - Causal masking: only key tiles j with 128*j < W*(qb+1) are computed; the
    two diagonal tiles get a multiplicative 0/1 mask after exp.
  - ST-matmul groups are software-pipelined up to 4 groups ahead of the PV
    matmuls (tapered at the end) so TensorE streams future scores while
    ScalarE exponentiates and TensorE consumes finished groups for PV;
    ScalarE (the exp bottleneck, ~1 elem/lane/cycle) stays saturated.
Output per core: outT [17, T] (16 unnormalized head dims + the exp-sum row).
Host divides and transposes during the gather step.
"""

import os

import numpy as np

B, T, D, H = 8, 2048, 128, 16
NT = T // 128        # 16 key tiles of 128
W = 256              # query block width (fp32r needs moving dim >= 256)
NQB = T // W         # 8 query blocks
GROUP = 4            # key tiles per exp call ([128, GROUP*W] = 2 PSUM banks)
SCALE = H ** -0.5

_CACHE = {}


def _build(prec: str):
    import concourse.mybir as mybir
    import concourse.tile as tile
    from concourse import bacc

    f32 = mybir.dt.float32
    mm_dt = {"f32r": mybir.dt.float32r, "f16": mybir.dt.float16, "f32": f32}[prec]
    Exp = mybir.ActivationFunctionType.Exp

    nc = bacc.Bacc()
    xT_d = nc.declare_dram_parameter("xT", [D, T], mm_dt, isOutput=False)
    # packed constants: wq[0:128] | wk[128:256] | wv[256:272]
    cst = nc.declare_dram_parameter("cst", [128, 272], mm_dt, isOutput=False)
    outT = nc.declare_dram_parameter("outT", [H + 1, T], f32, isOutput=True)

    with tile.TileContext(nc) as tc:
        with tc.tile_pool(name="sb", bufs=1) as sb:
            # ---- persistent SBUF buffers ----
            cst_sb = sb.tile([128, 272], mm_dt, tag="cst")
            nc.gpsimd.dma_start(cst_sb[:], cst.ap())  # SWDGE: parallel to x
            wq_sb = cst_sb[:, 0:128]
            wk_sb = cst_sb[:, 128:256]
            wv_sb = cst_sb[:, 256:272]
            # diagonal masks generated on the idle GPSIMD engine:
            # dm[:, c] over the two diagonal key tiles (see _host_inputs docs)
            mdt = mm_dt if prec == "f16" else f32
            dm_full = sb.tile([128, 2 * W], mdt, tag="dm")
            nc.gpsimd.memset(dm_full[:], 1.0)
            nc.gpsimd.affine_select(
                out=dm_full[:, :W], in_=dm_full[:, :W],
                compare_op=mybir.AluOpType.is_ge, fill=0.0,
                base=0, pattern=[[1, W]], channel_multiplier=-1,
            )
            nc.gpsimd.affine_select(
                out=dm_full[:, W:], in_=dm_full[:, W:],
                compare_op=mybir.AluOpType.is_ge, fill=0.0,
                base=-128, pattern=[[1, W]], channel_multiplier=-1,
            )
            dm_sb = dm_full

            CH = [(0, 256), (256, 256), (512, 512), (1024, 512), (1536, 512)]
            xT = sb.tile([128, T], mm_dt, tag="xT")           # [d, t]
            for c0, cw in CH:
                nc.sync.dma_start(
                    xT[:, c0:c0 + cw], xT_d.ap()[:, c0:c0 + cw]
                )

            warm = sb.tile([1, 2], f32, tag="warm")
            nc.vector.memset(warm[:, 0:1], 0.0)
            nc.scalar.activation(warm[:, 1:2], warm[:, 0:1], Exp)
            # PE warm-up: dummy matmuls during the input DMA keep the HAM
            # activity monitor busy so real matmuls start at full clock.
            wdum = sb.tile([128, 512], mm_dt, tag="wdum")
            if prec == "f32r":
                nc.vector.memset(wdum[:].bitcast(f32), 0.0)
            else:
                nc.vector.memset(wdum[:], 0.0)

            qTc = [sb.tile([128, cw], mm_dt, tag=f"qT{g}", name=f"qT{g}")
                   for g, (c0, cw) in enumerate(CH)]
            kTc = [sb.tile([128, cw], mm_dt, tag=f"kT{g}", name=f"kT{g}")
                   for g, (c0, cw) in enumerate(CH)]
            vSc = [sb.tile([128, cw // 128, H + 1], mm_dt, tag=f"vS{g}", name=f"vS{g}")
                   for g, (c0, cw) in enumerate(CH)]

            def chunk_of(col):  # chunk index, offset for column `col`
                for g, (c0, cw) in enumerate(CH):
                    if c0 <= col < c0 + cw:
                        return g, col - c0
                raise AssertionError(col)
            oTc = [sb.tile([H + 1, W], f32, tag=f"oT{qb}", name=f"oT{qb}") for qb in range(NQB)]

            groups = []  # (qb, nj, g0, gn, first_emitted, last_emitted)
            for qb in range(NQB):
                nj = (W * (qb + 1)) // 128
                qb_groups = [
                    [qb, nj, g0, min(GROUP, nj - g0)]
                    for g0 in range(0, nj, GROUP)
                ]
                # descending start: the diagonal (masked) group is consumed
                # first, keeping the exp->mask->PV chain off the qb tail
                qb_groups = list(reversed(qb_groups))
                if qb == NQB - 1 and qb_groups[-1][3] == GROUP:
                    # split the kernel's very last group so the final
                    # exp->PV->copy->DMA chain is 1 tile deep
                    q, n2, g0, gn = qb_groups.pop()
                    qb_groups += [[q, n2, g0 + 1, gn - 1], [q, n2, g0, 1]]
                for i3, g in enumerate(qb_groups):
                    groups.append((*g, i3 == 0, i3 == len(qb_groups) - 1))

            with (
                tc.tile_pool(name="psS", bufs=3, space="PSUM") as psS,
                tc.tile_pool(name="psO", bufs=2, space="PSUM") as psO,
                tc.tile_pool(name="pt", bufs=8) as ptp,
            ):
                o_tiles = {}
                pt_tiles = {}

                def emit_proj(g):
                    c0, cw = CH[g]
                    nt = cw // 128
                    sl = slice(c0, c0 + cw)
                    if 2 * cw + nt * H <= GROUP * W:
                        pp = psS.tile([128, GROUP * W], f32, tag="st", name=f"pp{g}")
                        pk, pq, pv = pp[:, :cw], pp[:, cw:2 * cw], pp[:, 2 * cw:2 * cw + nt * H]
                    else:
                        pp = psS.tile([128, GROUP * W], f32, tag="st", name=f"ppa{g}")
                        pp2 = psS.tile([128, GROUP * W], f32, tag="st", name=f"ppb{g}")
                        pk, pv = pp[:, :cw], pp[:, cw:cw + nt * H]
                        pq = pp2[:, :cw]
                    nc.tensor.matmul(pk, wk_sb[:], xT[:, sl])
                    if g <= 2:  # ramp phase: ACT has idle capacity
                        nc.scalar.copy(kTc[g][:], pk)
                    else:
                        nc.vector.tensor_copy(kTc[g][:], pk)
                    nc.tensor.matmul(pq, wq_sb[:], xT[:, sl])
                    nc.vector.tensor_copy(qTc[g][:], pq)
                    pvv = pv.rearrange("p (n h) -> p n h", n=nt)
                    for u in range(nt):
                        i = (c0 // 128) + u
                        nc.tensor.matmul(
                            pvv[:, u, :], xT[:, 128 * i:128 * (i + 1)], wv_sb[:]
                        )
                    nc.vector.tensor_copy(vSc[g][:, :, :H], pvv[:])
                    if prec == "f32r":
                        nc.vector.memset(vSc[g][:, :, H].bitcast(f32), 1.0)
                    else:
                        nc.vector.memset(vSc[g][:, :, H], 1.0)

                def q_ap(qb):
                    g, off = chunk_of(W * qb)
                    return qTc[g][:, off:off + W]

                def emit_st_exp(idx):
                    qb, nj, g0, gn, _first, _last = groups[idx]
                    st = psS.tile([128, GROUP * W], f32, tag="st")
                    for jj in range(gn):
                        j = g0 + jj
                        kg, koff = chunk_of(128 * j)
                        nc.tensor.matmul(
                            st[:, jj * W:(jj + 1) * W],
                            kTc[kg][:, koff:koff + 128],
                            q_ap(qb),
                        )
                    pt = ptp.tile([128, GROUP * W], mm_dt, tag="pt")
                    pt_tiles[idx] = pt
                    nc.scalar.activation(
                        pt[:, :gn * W], st[:, :gn * W], Exp, scale=SCALE
                    )
                    if g0 + gn == nj:  # group holding the 2 diagonal tiles
                        off = (nj - 2 - g0) * W
                        nc.vector.tensor_mul(
                            pt[:, off:off + 2 * W], pt[:, off:off + 2 * W], dm_sb[:]
                        )

                def emit_pv(idx):
                    qb, nj, g0, gn, first_emitted, last_emitted = groups[idx]
                    if first_emitted:
                        o_tiles[qb] = psO.tile([H + 1, W], f32, tag="o", name=f"o{qb}")
                    pt = pt_tiles.pop(idx)
                    for jj in range(gn):
                        j = g0 + jj
                        vg, voff = chunk_of(128 * j)
                        nc.tensor.matmul(
                            o_tiles[qb][:],
                            vSc[vg][:, voff // 128, :],
                            pt[:, jj * W:(jj + 1) * W],
                            start=(first_emitted and jj == 0),
                            stop=(last_emitted and jj == gn - 1),
                        )
                    if last_emitted:
                        nc.vector.tensor_copy(oTc[qb][:], o_tiles.pop(qb)[:])
                        nc.sync.dma_start(
                            outT.ap()[:, W * qb:W * (qb + 1)], oTc[qb][:]
                        )

                # projection-chunk emission slots (group indices). Chunks
                # MUST be emitted at or before the first group that reads
                # them: Tile tracks dependencies by trace order, so a
                # consumer emitted before its producer silently reads stale
                # SBUF (verified: gives nondeterministic garbage).
                proj_at = {0: [0, 1], 1: [2], 2: [3], 6: [4]}
                first_need = {}
                for i2, (qb, nj, g0, gn, _f, _l) in enumerate(groups):
                    need = {chunk_of(W * qb)[0]}
                    need.update(chunk_of(128 * j)[0] for j in range(g0, g0 + gn))
                    for g in need:
                        first_need.setdefault(g, i2)
                for slot, gs in proj_at.items():
                    for g in gs:
                        assert slot <= first_need[g], (slot, g, first_need[g])

                n = len(groups)
                pdum = psS.tile([128, GROUP * W], f32, tag="st", name="pdum")
                for r in range(4):
                    nc.tensor.matmul(pdum[:, :512], wdum[:, :128], wdum[:])
                for g in proj_at.pop(0, []):
                    emit_proj(g)
                pend = []
                for idx in range(n):
                    for g in proj_at.pop(idx, []):
                        emit_proj(g)
                    emit_st_exp(idx)
                    pend.append(idx)
                    depth = 4 if idx < n - 4 else max(1, n - 1 - idx)
                    while len(pend) > depth:
                        emit_pv(pend.pop(0))
                while pend:
                    emit_pv(pend.pop(0))

    nc.finalize()
    return nc


def _get_nc(prec: str):
    if prec not in _CACHE:
        _CACHE[prec] = _build(prec)
    return _CACHE[prec]


def _host_inputs(Wq, Wk, Wv):
    Wq, Wk, Wv = (np.asarray(w, dtype=np.float32) for w in (Wq, Wk, Wv))
    cst = np.zeros((128, 272), np.float32)
    cst[:, 0:H] = Wq
    cst[:, 128:128 + H] = Wk
    cst[:D, 256:256 + H] = Wv
    return cst


def kernel(inpEmb, Wq, Wk, Wv):
    from concourse.bass_utils import run_bass_kernel_spmd

    prec = os.environ.get("ATT_PREC", "f32r")
    nc = _get_nc(prec)
    np_dt = np.float16 if prec == "f16" else np.float32
    cst = _host_inputs(Wq, Wk, Wv).astype(np_dt)
    x = np.asarray(inpEmb, dtype=np.float32)
    in_maps = [
        {"xT": np.ascontiguousarray(x[b].T.astype(np_dt)), "cst": cst}
        for b in range(B)
    ]
    def run_and_check():
        br = run_bass_kernel_spmd(nc, in_maps, list(range(B)))
        out = np.empty((B, T, H), np.float32)
        for b in range(B):
            oT = br.results[b]["outT"]
            sums = oT[H]
            # softmax denominators are sums of exponentials: must be finite
            # and strictly positive; anything else means the device run was
            # bad (unwritten/partial output) and should be retried.
            if not (np.isfinite(oT).all() and (sums > 0.0).all()):
                raise RuntimeError(f"core {b}: invalid kernel output")
            out[b] = (oT[:H] / sums[None, :]).T
        return out

    for attempt in range(3):
        try:
            return run_and_check()
        except Exception:
            if attempt == 2:
                raise



# revision 68
# speedup vs baseline: 1.0094x; 1.0052x over previous
"""Causal single-head attention (B=8, T=2048, D=128, H=16) on 8 Trainium2 cores.

Strategy: data-parallel over batch (1 batch element per NeuronCore). Per core:
  - x arrives pre-transposed from the host as xT [D, T] (contiguous DMA).
  - Project qT/kT = (Wq/Wk)^T @ xT with head dim zero-padded 16->128 so every
    matmul contracts over K=128; v tiles [128, 17] carry a ones column so the
    softmax denominator falls out of the PV matmul for free.
  - Scores are computed TRANSPOSED: ST[keys, queries] = kT_j^T @ qT_block, so
    exp(ST) (ACT, scale=1/4 folded in) is directly the PV stationary-side
    operand -- no per-tile transposes of the probability matrix.
  - PV: O'T[17, W] += V'_j^T @ PT_j accumulated in PSUM over key tiles.
  - Causal masking: only key tiles j with 128*j < W*(qb+1) are computed; the
    two diagonal tiles get a multiplicative 0/1 mask after exp.
  - ST-matmul groups are software-pipelined up to 4 groups ahead of the PV
    matmuls (tapered at the end) so TensorE streams future scores while
    ScalarE exponentiates and TensorE consumes finished groups for PV;
    ScalarE (the exp bottleneck, ~1 elem/lane/cycle) stays saturated.
Output per core: outT [17, T] (16 unnormalized head dims + the exp-sum row).
Host divides and transposes during the gather step.
"""

import os

import numpy as np

B, T, D, H = 8, 2048, 128, 16
NT = T // 128        # 16 key tiles of 128
W = 256              # query block width (fp32r needs moving dim >= 256)
NQB = T // W         # 8 query blocks
GROUP = 4            # key tiles per exp call ([128, GROUP*W] = 2 PSUM banks)
SCALE = H ** -0.5

_CACHE = {}


def _build(prec: str):
    import concourse.mybir as mybir
    import concourse.tile as tile
    from concourse import bacc

    f32 = mybir.dt.float32
    mm_dt = {"f32r": mybir.dt.float32r, "f16": mybir.dt.float16, "f32": f32}[prec]
    Exp = mybir.ActivationFunctionType.Exp

    nc = bacc.Bacc()
    xT_d = nc.declare_dram_parameter("xT", [D, T], mm_dt, isOutput=False)
    # packed constants: wq[0:128] | wk[128:256] | wv[256:272]
    cst = nc.declare_dram_parameter("cst", [128, 272], mm_dt, isOutput=False)
    outT = nc.declare_dram_parameter("outT", [H + 1, T], f32, isOutput=True)

    with tile.TileContext(nc) as tc:
        with tc.tile_pool(name="sb", bufs=1) as sb:
            # ---- persistent SBUF buffers ----
            cst_sb = sb.tile([128, 272], mm_dt, tag="cst")
            nc.gpsimd.dma_start(cst_sb[:], cst.ap())  # SWDGE: parallel to x
            wq_sb = cst_sb[:, 0:128]
            wk_sb = cst_sb[:, 128:256]
            wv_sb = cst_sb[:, 256:272]
            # diagonal masks generated on the idle GPSIMD engine:
            # dm[:, c] over the two diagonal key tiles (see _host_inputs docs)
            mdt = mm_dt if prec == "f16" else f32
            dm_full = sb.tile([128, 2 * W], mdt, tag="dm")
            nc.gpsimd.memset(dm_full[:], 1.0)
            nc.gpsimd.affine_select(
                out=dm_full[:, :W], in_=dm_full[:, :W],
                compare_op=mybir.AluOpType.is_ge, fill=0.0,
                base=0, pattern=[[1, W]], channel_multiplier=-1,
            )
            nc.gpsimd.affine_select(
                out=dm_full[:, W:], in_=dm_full[:, W:],
                compare_op=mybir.AluOpType.is_ge, fill=0.0,
                base=-128, pattern=[[1, W]], channel_multiplier=-1,
            )
            dm_sb = dm_full

            CH = [(0, 256), (256, 256), (512, 512), (1024, 512), (1536, 512)]
            xT = sb.tile([128, T], mm_dt, tag="xT")           # [d, t]
            for c0, cw in CH:
                nc.sync.dma_start(
                    xT[:, c0:c0 + cw], xT_d.ap()[:, c0:c0 + cw]
                )

            warm = sb.tile([1, 2], f32, tag="warm")
            nc.vector.memset(warm[:, 0:1], 0.0)
            nc.scalar.activation(warm[:, 1:2], warm[:, 0:1], Exp)
            # PE warm-up: dummy matmuls during the input DMA keep the HAM
            # activity monitor busy so real matmuls start at full clock.
            wdum = sb.tile([128, 512], mm_dt, tag="wdum")
            if prec == "f32r":
                nc.vector.memset(wdum[:].bitcast(f32), 0.0)
            else:
                nc.vector.memset(wdum[:], 0.0)

            qTc = [sb.tile([128, cw], mm_dt, tag=f"qT{g}", name=f"qT{g}")
                   for g, (c0, cw) in enumerate(CH)]
            kTc = [sb.tile([128, cw], mm_dt, tag=f"kT{g}", name=f"kT{g}")
                   for g, (c0, cw) in enumerate(CH)]
            vSc = [sb.tile([128, cw // 128, H + 1], mm_dt, tag=f"vS{g}", name=f"vS{g}")
                   for g, (c0, cw) in enumerate(CH)]

            def chunk_of(col):  # chunk index, offset for column `col`
                for g, (c0, cw) in enumerate(CH):
                    if c0 <= col < c0 + cw:
                        return g, col - c0
                raise AssertionError(col)
            oTc = [sb.tile([H + 1, W], f32, tag=f"oT{qb}", name=f"oT{qb}") for qb in range(NQB)]

            groups = []  # (qb, nj, g0, gn, first_emitted, last_emitted)
            for qb in range(NQB):
                nj = (W * (qb + 1)) // 128
                qb_groups = [
                    [qb, nj, g0, min(GROUP, nj - g0)]
                    for g0 in range(0, nj, GROUP)
                ]
                # descending start: the diagonal (masked) group is consumed
                # first, keeping the exp->mask->PV chain off the qb tail
                qb_groups = list(reversed(qb_groups))
                if qb == NQB - 1 and qb_groups[-1][3] == GROUP:
                    # split the kernel's very last group so the final
                    # exp->PV->copy->DMA chain is 1 tile deep
                    q, n2, g0, gn = qb_groups.pop()
                    qb_groups += [[q, n2, g0 + 1, gn - 1], [q, n2, g0, 1]]
                for i3, g in enumerate(qb_groups):
                    groups.append((*g, i3 == 0, i3 == len(qb_groups) - 1))

            with (
                tc.tile_pool(name="psS", bufs=3, space="PSUM") as psS,
                tc.tile_pool(name="psO", bufs=2, space="PSUM") as psO,
                tc.tile_pool(name="pt", bufs=8) as ptp,
            ):
                o_tiles = {}
                pt_tiles = {}

                def emit_proj(g):
                    c0, cw = CH[g]
                    nt = cw // 128
                    sl = slice(c0, c0 + cw)
                    if 2 * cw + nt * H <= GROUP * W:
                        pp = psS.tile([128, GROUP * W], f32, tag="st", name=f"pp{g}")
                        pk, pq, pv = pp[:, :cw], pp[:, cw:2 * cw], pp[:, 2 * cw:2 * cw + nt * H]
                    else:
                        pp = psS.tile([128, GROUP * W], f32, tag="st", name=f"ppa{g}")
                        pp2 = psS.tile([128, GROUP * W], f32, tag="st", name=f"ppb{g}")
                        pk, pv = pp[:, :cw], pp[:, cw:cw + nt * H]
                        pq = pp2[:, :cw]
                    nc.tensor.matmul(pk, wk_sb[:], xT[:, sl])
                    if g in (1, 2):  # ramp: ACT idle; chunk 0 stays on DVE
                        nc.scalar.copy(kTc[g][:], pk)  # (ACT queue at chunk 0
                    else:  # time is still busy with the exp-table load)
                        nc.vector.tensor_copy(kTc[g][:], pk)
                    nc.tensor.matmul(pq, wq_sb[:], xT[:, sl])
                    nc.vector.tensor_copy(qTc[g][:], pq)
                    pvv = pv.rearrange("p (n h) -> p n h", n=nt)
                    for u in range(nt):
                        i = (c0 // 128) + u
                        nc.tensor.matmul(
                            pvv[:, u, :], xT[:, 128 * i:128 * (i + 1)], wv_sb[:]
                        )
                    nc.vector.tensor_copy(vSc[g][:, :, :H], pvv[:])
                    if prec == "f32r":
                        nc.vector.memset(vSc[g][:, :, H].bitcast(f32), 1.0)
                    else:
                        nc.vector.memset(vSc[g][:, :, H], 1.0)

                def q_ap(qb):
                    g, off = chunk_of(W * qb)
                    return qTc[g][:, off:off + W]

                def emit_st_exp(idx):
                    qb, nj, g0, gn, _first, _last = groups[idx]
                    st = psS.tile([128, GROUP * W], f32, tag="st")
                    for jj in range(gn):
                        j = g0 + jj
                        kg, koff = chunk_of(128 * j)
                        nc.tensor.matmul(
                            st[:, jj * W:(jj + 1) * W],
                            kTc[kg][:, koff:koff + 128],
                            q_ap(qb),
                        )
                    pt = ptp.tile([128, GROUP * W], mm_dt, tag="pt")
                    pt_tiles[idx] = pt
                    nc.scalar.activation(
                        pt[:, :gn * W], st[:, :gn * W], Exp, scale=SCALE
                    )
                    if g0 + gn == nj:  # group holding the 2 diagonal tiles
                        off = (nj - 2 - g0) * W
                        nc.vector.tensor_mul(
                            pt[:, off:off + 2 * W], pt[:, off:off + 2 * W], dm_sb[:]
                        )

                def emit_pv(idx):
                    qb, nj, g0, gn, first_emitted, last_emitted = groups[idx]
                    if first_emitted:
                        o_tiles[qb] = psO.tile([H + 1, W], f32, tag="o", name=f"o{qb}")
                    pt = pt_tiles.pop(idx)
                    for jj in range(gn):
                        j = g0 + jj
                        vg, voff = chunk_of(128 * j)
                        nc.tensor.matmul(
                            o_tiles[qb][:],
                            vSc[vg][:, voff // 128, :],
                            pt[:, jj * W:(jj + 1) * W],
                            start=(first_emitted and jj == 0),
                            stop=(last_emitted and jj == gn - 1),
                        )
                    if last_emitted:
                        nc.vector.tensor_copy(oTc[qb][:], o_tiles.pop(qb)[:])
                        nc.sync.dma_start(
                            outT.ap()[:, W * qb:W * (qb + 1)], oTc[qb][:]
                        )

                # projection-chunk emission slots (group indices). Chunks
                # MUST be emitted at or before the first group that reads
                # them: Tile tracks dependencies by trace order, so a
                # consumer emitted before its producer silently reads stale
                # SBUF (verified: gives nondeterministic garbage).
                proj_at = {0: [0, 1], 1: [2], 2: [3], 6: [4]}
                first_need = {}
                for i2, (qb, nj, g0, gn, _f, _l) in enumerate(groups):
                    need = {chunk_of(W * qb)[0]}
                    need.update(chunk_of(128 * j)[0] for j in range(g0, g0 + gn))
                    for g in need:
                        first_need.setdefault(g, i2)
                for slot, gs in proj_at.items():
                    for g in gs:
                        assert slot <= first_need[g], (slot, g, first_need[g])

                n = len(groups)
                pdum = psS.tile([128, GROUP * W], f32, tag="st", name="pdum")
                for r in range(4):
                    nc.tensor.matmul(pdum[:, :512], wdum[:, :128], wdum[:])
                for g in proj_at.pop(0, []):
                    emit_proj(g)
                pend = []
                for idx in range(n):
                    for g in proj_at.pop(idx, []):
                        emit_proj(g)
                    emit_st_exp(idx)
                    pend.append(idx)
                    depth = 4 if idx < n - 4 else max(1, n - 1 - idx)
                    while len(pend) > depth:
                        emit_pv(pend.pop(0))
                while pend:
                    emit_pv(pend.pop(0))

    nc.finalize()
    return nc


def _get_nc(prec: str):
    if prec not in _CACHE:
        _CACHE[prec] = _build(prec)
    return _CACHE[prec]


def _host_inputs(Wq, Wk, Wv):
    Wq, Wk, Wv = (np.asarray(w, dtype=np.float32) for w in (Wq, Wk, Wv))
    cst = np.zeros((128, 272), np.float32)
    cst[:, 0:H] = Wq
    cst[:, 128:128 + H] = Wk
    cst[:D, 256:256 + H] = Wv
    return cst


def kernel(inpEmb, Wq, Wk, Wv):
    from concourse.bass_utils import run_bass_kernel_spmd

    prec = os.environ.get("ATT_PREC", "f32r")
    nc = _get_nc(prec)
    np_dt = np.float16 if prec == "f16" else np.float32
    cst = _host_inputs(Wq, Wk, Wv).astype(np_dt)
    x = np.asarray(inpEmb, dtype=np.float32)
    in_maps = [
        {"xT": np.ascontiguousarray(x[b].T.astype(np_dt)), "cst": cst}
        for b in range(B)
    ]
    def run_and_check():
        br = run_bass_kernel_spmd(nc, in_maps, list(range(B)))
        out = np.empty((B, T, H), np.float32)
        for b in range(B):
            oT = br.results[b]["outT"]
            sums = oT[H]
            # softmax denominators are sums of exponentials: must be finite
            # and strictly positive; anything else means the device run was
            # bad (unwritten/partial output) and should be retried.
            if not (np.isfinite(oT).all() and (sums > 0.0).all()):
                raise RuntimeError(f"core {b}: invalid kernel output")
            out[b] = (oT[:H] / sums[None, :]).T
        return out

    for attempt in range(3):
        try:
            return run_and_check()
        except Exception:
            if attempt == 2:
                raise

